# revision 1
# baseline (speedup 1.0000x reference)
"""Trainium2 Bass kernel for nn_DfOpCoefLoop (deep-filter complex FIR + alpha blend).

Reference semantics (per batch b, time t, freq bin f < 96):
    spec_f[t,f] = sum_{i=0..4} x[t+i-2, f] * coefs[t,i,f]      (complex MAC, zero-padded in t)
    out[t,f]    = alpha[t] * spec_f[t,f] + (1-alpha[t]) * x[t,f]
    out[t,f]    = spec[t,f]                                    (f >= 96 passthrough)

Strategy: pure data-parallel over batch (32 batches -> 8 cores x 4 batches).

The host pre-arranges every input in the exact order the engines consume it, so
the device program is ~40 large DMAs plus a few hundred contiguous compute ops:
  - X5: the 5 time-shifted windows, deinterleaved and stacked per t-row as
    [xr taps (5,96) | xi taps (5,96)], zero-padded at the t edges, then blocked
    to (partition = t%128, chunk = t//128).
  - CX: coefs per t-row as [cr (5,96) | -ci (5,96)] (ci pre-negated), blocked
    the same way.
  - alpha / (1-alpha) as per-partition scalar tables [128, batch*chunk].
Compute runs in 4-chunk groups (tensor_reduce has ~1us fixed cost, so reduces
are merged across chunks):
    m1 = X5*CX = [xr*cr | xi*(-ci)]   --one 10-tap reduce--> re     (DVE)
    m2a = xi*cr (TT), m2b = (xr*-1)*(-ci) = xr*ci (STT imm)         (GPSIMD)
                                      --one 10-tap reduce--> im     (DVE)
    out = alpha*(re|im) + (1-alpha)*x0   (per-chunk STT; (1-a)*x0 on ScalarE)
The f>=96 bins never touch the device: the host copies them straight from the
input when assembling the full output (identity passthrough).
"""

import numpy as np

ORDER = 5
LOOKAHEAD = 2
F = 96            # deep-filtered bins
FC = 2 * F        # one t-row of (c,f) planar data: 192 floats
HB = ORDER * F    # 480: one comp block of taps
W = ORDER * FC    # 960: stacked taps [xr5 | xi5] / coef row [cr5 | mci5]
NFREQ = 481
B, T = 32, 1000
NCORES = 8
BPC = B // NCORES  # batches per core
GRP = 4            # chunks per compute group

_CACHE = {}


def _build_program(bpc, t_len):
    """Build the per-core Bass program (returns a compiled Bacc)."""
    import concourse.bacc as bacc
    import concourse.mybir as mybir
    import concourse.tile as tile

    nk = (t_len + 127) // 128          # time chunks per batch
    assert nk % GRP == 0
    ncols = bpc * nk                   # alpha table columns
    GW = GRP * W                       # group free-dim span (3840)

    # Bacc (not raw Bass): its compile() runs generate_event_semaphores,
    # which splits multi-wait sync onto EventSemaphore instructions --
    # TRN2 instructions encode at most one sem wait.
    nc = bacc.Bacc("TRN2", target_bir_lowering=False, debug=False)
    dt = mybir.dt.float32

    x5_t = nc.dram_tensor("x5_t", [bpc, 128, nk * W], dt, kind="ExternalInput").ap()
    cx_t = nc.dram_tensor("cx_t", [bpc, 128, nk * W], dt, kind="ExternalInput").ap()
    alpha_t = nc.dram_tensor("alpha_t", [128, ncols], dt, kind="ExternalInput").ap()
    oma_t = nc.dram_tensor("oma_t", [128, ncols], dt, kind="ExternalInput").ap()
    outb = nc.dram_tensor("outb", [bpc, 128, nk * FC], dt, kind="ExternalOutput").ap()

    mul = mybir.AluOpType.mult
    add = mybir.AluOpType.add
    copy_fn = mybir.ActivationFunctionType.Copy

    def jview(t, off, run):
        """(j, run)-strided view of a group tile: j-stride W, GRP rows."""
        return t.rearrange("p (j w) -> p j w", j=GRP, w=W)[:, :, off : off + run]

    with tile.TileContext(nc) as tc:
        with (
            tc.tile_pool(name="const", bufs=1) as const_pool,
            tc.tile_pool(name="x5g", bufs=2) as x5_pool,
            tc.tile_pool(name="cxg", bufs=2) as cx_pool,
            tc.tile_pool(name="obp", bufs=2) as ob_pool,
            tc.tile_pool(name="p1", bufs=2) as p1_pool,
            tc.tile_pool(name="p2", bufs=2) as p2_pool,
            tc.tile_pool(name="small", bufs=3) as small_pool,
        ):
            alpha_sb = const_pool.tile([128, ncols], dt, name="alpha_sb")
            oma_sb = const_pool.tile([128, ncols], dt, name="oma_sb")
            nc.sync.dma_start(alpha_sb[:], alpha_t[:])
            nc.sync.dma_start(oma_sb[:], oma_t[:])

            for b in range(bpc):
                ob = ob_pool.tile([128, nk * FC], dt, name="ob")
                for g in range(nk // GRP):
                    gs = g * GW
                    x5g = x5_pool.tile([128, GW], dt, name="x5g")
                    cxg = cx_pool.tile([128, GW], dt, name="cxg")
                    nc.sync.dma_start(x5g[:], x5_t[b, :, gs : gs + GW])
                    nc.scalar.dma_start(cxg[:], cx_t[b, :, gs : gs + GW])

                    p1 = p1_pool.tile([128, GW], dt, name="p1")
                    p2 = p2_pool.tile([128, GW], dt, name="p2")
                    acc = small_pool.tile([128, GRP * FC], dt, name="acc")
                    v = small_pool.tile([128, GRP * FC], dt, name="v")

                    # m1 = X5 * CX (fully contiguous); alternate engine
                    m1_eng = nc.vector if (b + g) % 2 == 0 else nc.gpsimd
                    m1_eng.tensor_mul(p1[:], x5g[:], cxg[:])
                    # re = sum of the 10 (comp,tap) products
                    nc.vector.tensor_reduce(
                        acc[:].rearrange("p (j c f) -> p j c f", j=GRP, c=2, f=F)[
                            :, :, 0:1
                        ].squeeze(2),
                        p1[:].rearrange(
                            "p (j gi f) -> p j f gi", j=GRP, gi=2 * ORDER, f=F
                        ),
                        axis=mybir.AxisListType.X,
                        op=add,
                    )
                    # m2a = xi*cr (GPSIMD) ; m2b = (xr*-1)*(-ci) = xr*ci (DVE STT)
                    nc.gpsimd.tensor_mul(
                        jview(p2, 0, HB), jview(x5g, HB, HB), jview(cxg, 0, HB)
                    )
                    nc.vector.scalar_tensor_tensor(
                        jview(p2, HB, HB),
                        jview(x5g, 0, HB),
                        -1.0,
                        jview(cxg, HB, HB),
                        op0=mul,
                        op1=mul,
                    )
                    # im = sum of the 10 products
                    nc.vector.tensor_reduce(
                        acc[:].rearrange("p (j c f) -> p j c f", j=GRP, c=2, f=F)[
                            :, :, 1:2
                        ].squeeze(2),
                        p2[:].rearrange(
                            "p (j gi f) -> p j f gi", j=GRP, gi=2 * ORDER, f=F
                        ),
                        axis=mybir.AxisListType.X,
                        op=add,
                    )
                    # blend per chunk (alpha is a per-(b,chunk) partition scalar)
                    for kk in range(GRP):
                        col = b * nk + g * GRP + kk
                        # v = (1-alpha) * x0 ; x0 = tap d=0 of X5 (planar view)
                        nc.scalar.activation(
                            v[:, kk * FC : (kk + 1) * FC].rearrange(
                                "p (c f) -> p c f", c=2, f=F
                            ),
                            x5g[:, kk * W : (kk + 1) * W]
                            .rearrange("p (c i f) -> p c i f", c=2, i=ORDER, f=F)[
                                :, :, LOOKAHEAD : LOOKAHEAD + 1
                            ]
                            .squeeze(2),
                            copy_fn,
                            scale=oma_sb[:, col : col + 1],
                        )
                        # out = alpha*acc + v
                        nc.vector.scalar_tensor_tensor(
                            ob[:, (g * GRP + kk) * FC : (g * GRP + kk + 1) * FC],
                            acc[:, kk * FC : (kk + 1) * FC],
                            alpha_sb[:, col : col + 1],
                            v[:, kk * FC : (kk + 1) * FC],
                            op0=mul,
                            op1=add,
                        )

                nc.sync.dma_start(outb[b], ob[:])
    nc.compile()
    return nc


def _get_program(bpc=BPC, t_len=T):
    key = (bpc, t_len)
    if key not in _CACHE:
        _CACHE[key] = _build_program(bpc, t_len)
    return _CACHE[key]


def _block(a, nk):
    """(nk*128, R) -> [128, nk*R] with partition = t%128."""
    n, r = a.shape
    assert n == nk * 128
    return np.ascontiguousarray(
        a.reshape(nk, 128, r).transpose(1, 0, 2).reshape(128, nk * r)
    )


def _host_prep(spec, coefs, alpha, bpc, t_len):
    """Re-layout one core's inputs into the device consumption order."""
    nk = (t_len + 127) // 128
    tp = nk * 128
    spec2 = np.asarray(spec[:, 0], dtype=np.float32)          # (bpc, t, 481, 2)
    xr = spec2[:, :, :F, 0]                                    # (bpc, t, 96)
    xi = spec2[:, :, :F, 1]
    xrp = np.zeros((bpc, tp + ORDER - 1, F), np.float32)
    xip = np.zeros((bpc, tp + ORDER - 1, F), np.float32)
    xrp[:, LOOKAHEAD : LOOKAHEAD + t_len] = xr
    xip[:, LOOKAHEAD : LOOKAHEAD + t_len] = xi
    # taps: X5[t, i, f] = x[t + i - LOOKAHEAD]
    xr5 = np.stack([xrp[:, i : i + tp] for i in range(ORDER)], axis=2)  # (bpc,tp,5,96)
    xi5 = np.stack([xip[:, i : i + tp] for i in range(ORDER)], axis=2)
    x5 = np.concatenate(
        [xr5.reshape(bpc, tp, HB), xi5.reshape(bpc, tp, HB)], axis=2
    )                                                          # (bpc, tp, 960)

    cr = np.asarray(coefs[..., 0], dtype=np.float32).reshape(bpc, t_len, HB)
    ci = np.asarray(coefs[..., 1], dtype=np.float32).reshape(bpc, t_len, HB)
    cx = np.zeros((bpc, tp, W), np.float32)
    cx[:, :t_len, :HB] = cr
    cx[:, :t_len, HB:] = -ci

    x5_t = np.stack([_block(x5[b], nk) for b in range(bpc)])
    cx_t = np.stack([_block(cx[b], nk) for b in range(bpc)])

    al = np.zeros((bpc, tp), np.float32)
    al[:, :t_len] = alpha[:, :, 0]
    alpha_t = np.ascontiguousarray(
        al.reshape(bpc, nk, 128).transpose(2, 0, 1).reshape(128, bpc * nk)
    )
    oma_t = np.ascontiguousarray(1.0 - alpha_t)
    return {
        "x5_t": x5_t,
        "cx_t": cx_t,
        "alpha_t": alpha_t,
        "oma_t": oma_t,
    }


def _unblock_out(ob, t_len):
    """[128, nk*192] planar (c,f) blocked -> (t, 96, 2) interleaved."""
    nk = ob.shape[1] // FC
    a = ob.reshape(128, nk, 2, F).transpose(1, 0, 2, 3).reshape(nk * 128, 2, F)
    return np.ascontiguousarray(a[:t_len].transpose(0, 2, 1))  # (t, 96, 2)


def run_on_cores(spec, coefs, alpha, trace=False):
    """Full-input entry: shard, run on 8 cores, return (out_full, results_obj)."""
    from concourse import bass_utils

    nc = _get_program()
    in_maps = [
        _host_prep(
            spec[c * BPC : (c + 1) * BPC],
            coefs[c * BPC : (c + 1) * BPC],
            alpha[c * BPC : (c + 1) * BPC],
            BPC,
            T,
        )
        for c in range(NCORES)
    ]
    res = bass_utils.run_bass_kernel_spmd(
        nc, in_maps, core_ids=list(range(NCORES)), trace=trace
    )
    full = np.array(spec, dtype=np.float32, copy=True)  # f>=96 passthrough on host
    for c in range(NCORES):
        ob = res.results[c]["outb"]
        for b in range(BPC):
            full[c * BPC + b, 0, :, :F, :] = _unblock_out(ob[b], T)
    return full, res


def kernel(spec, coefs, alpha):
    spec = np.asarray(spec, dtype=np.float32)
    coefs = np.asarray(coefs, dtype=np.float32)
    alpha = np.asarray(alpha, dtype=np.float32)
    full, _ = run_on_cores(spec, coefs, alpha, trace=False)
    return full



# revision 3
# speedup vs baseline: 5.9788x; 5.9788x over previous
"""Trainium2 Bass kernel for nn_DfOpCoefLoop (deep-filter complex FIR + alpha blend).

Reference semantics (per batch b, time t, freq bin f < 96):
    spec_f[t,f] = sum_{i=0..4} x[t+i-2, f] * coefs[t,i,f]      (complex MAC, zero-padded in t)
    out[t,f]    = alpha[t] * spec_f[t,f] + (1-alpha[t]) * x[t,f]
    out[t,f]    = spec[t,f]                                    (f >= 96 passthrough)

The 8 NeuronCores are axon-tunneled: host<->device bytes move at ~80MB/s, so the
end-to-end time is dominated by wire traffic.  Strategy:

  - pure data parallel over batch (32 batches -> 8 cores x 4 batches)
  - ship the MINIMUM bytes, all fp16 (rel tol is 2e-2; fp16 end-to-end sims at
    7e-4): x unduplicated (12.6MB), coefs in their natural layout (63MB),
    alpha as tiny per-(b,chunk) partition-scalar tables.  No 5-tap host
    expansion, no deinterleave, no host blocking - the device does all of it.
  - donated output zero-buffers are created ON DEVICE (jnp.zeros), outputs
    come back fp16.
  - the jitted shard_map executable is cached module-level, so repeat calls
    skip retrace/recompile.

Device program (per core, per batch, per 128-row time chunk k):
  X5 (128,960) <- one DMA with an overlapping access pattern over the padded
      x rows: partition p gets rows [k*128+p .. k*128+p+4] (5 taps, 1920B
      contiguous per partition).  Slot i holds x[t+i-2] as (f,c) interleaved.
  C  (128,960) <- coefs rows, natural (i,f,c) layout, aligned with X5 slots.
  P1 = X5*C             -> [xr*cr at c=0 | xi*ci at c=1]
  Sre = reduce_i(P1)    -> (128,192) f32;  re = Sre[even] - Sre[odd]
  P2[even] = X5[odd]*C[even] (xi*cr),  P2[odd] = X5[even]*C[odd] (xr*ci)
  Sim = reduce_i(P2)    -> im = Sim[even] + Sim[odd]
  acc (128,192) f32 interleaved [re|im]
  out = alpha[col]*acc + (1-alpha[col])*x0   (x0 = X5 tap 2; per-partition
      scalar columns, tensor_scalar + scalar_tensor_tensor)
The f>=96 bins never touch the device: host copies them straight through.
"""

import dataclasses
import sys

import numpy as np

try:
    import concourse  # noqa: F401
except ImportError:
    sys.path.insert(0, "/opt/trn_rl_repo")

ORDER = 5
LOOKAHEAD = 2
F = 96            # deep-filtered bins
FC = 2 * F        # one t-row of interleaved (f,c): 192
W = ORDER * FC    # 960: one coefs row / 5 stacked taps
B, T = 32, 1000
NCORES = 8
BPC = B // NCORES  # batches per core
NK = 8             # time chunks of 128 per batch
TP = NK * 128      # 1024
XROWS = TP + ORDER - 1  # 1028: padded x rows, row r = x[t=r-2]

_CACHE = {}


def _build_program():
    """Build + compile the per-core Bass program."""
    import concourse.bacc as bacc
    import concourse.mybir as mybir
    import concourse.tile as tile

    nc = bacc.Bacc("TRN2", target_bir_lowering=False, debug=False)
    f16 = mybir.dt.float16
    f32 = mybir.dt.float32
    ncols = BPC * NK

    x_t = nc.dram_tensor("x_t", [BPC, XROWS, FC], f16, kind="ExternalInput").ap()
    c_t = nc.dram_tensor("c_t", [BPC, TP, W], f16, kind="ExternalInput").ap()
    alpha_t = nc.dram_tensor("alpha_t", [128, ncols], f32, kind="ExternalInput").ap()
    oma_t = nc.dram_tensor("oma_t", [128, ncols], f32, kind="ExternalInput").ap()
    outb = nc.dram_tensor("outb", [BPC, TP, FC], f16, kind="ExternalOutput").ap()

    mul = mybir.AluOpType.mult
    add = mybir.AluOpType.add
    sub = mybir.AluOpType.subtract

    def tap5(b, k):
        """Overlapping (128,5,192) view of x_t[b]: partition p -> rows k*128+p+i."""
        base = x_t[b]
        return dataclasses.replace(
            base,
            offset=base.offset + (k * 128) * FC,
            ap=[[FC, 128], [FC, ORDER], [1, FC]],
        )

    with tile.TileContext(nc) as tc:
        with (
            tc.tile_pool(name="const", bufs=1) as const_pool,
            tc.tile_pool(name="x5p", bufs=3) as x5_pool,
            tc.tile_pool(name="cp", bufs=3) as c_pool,
            tc.tile_pool(name="p1p", bufs=2) as p1_pool,
            tc.tile_pool(name="p2p", bufs=2) as p2_pool,
            tc.tile_pool(name="sm", bufs=3) as sm_pool,
            tc.tile_pool(name="obp", bufs=2) as ob_pool,
        ):
            alpha_sb = const_pool.tile([128, ncols], f32, name="alpha_sb")
            oma_sb = const_pool.tile([128, ncols], f32, name="oma_sb")
            nc.sync.dma_start(alpha_sb[:], alpha_t[:])
            nc.sync.dma_start(oma_sb[:], oma_t[:])

            for b in range(BPC):
                ob = ob_pool.tile([128, NK * FC], f16, name="ob")
                for k in range(NK):
                    col = b * NK + k
                    x5 = x5_pool.tile([128, W], f16, name="x5")
                    c = c_pool.tile([128, W], f16, name="c")
                    nc.sync.dma_start(x5[:], tap5(b, k))
                    nc.scalar.dma_start(c[:], c_t[b, k * 128 : (k + 1) * 128, :])

                    p1 = p1_pool.tile([128, W], f16, name="p1")
                    p2 = p2_pool.tile([128, W], f16, name="p2")
                    sre = sm_pool.tile([128, FC], f32, name="sre")
                    sim = sm_pool.tile([128, FC], f32, name="sim")
                    acc = sm_pool.tile([128, FC], f32, name="acc")
                    v = sm_pool.tile([128, FC], f32, name="v")

                    # interleaved (i,f,c) views
                    x5v = x5[:].rearrange("p (i f c) -> p i f c", i=ORDER, f=F, c=2)
                    cv = c[:].rearrange("p (i f c) -> p i f c", i=ORDER, f=F, c=2)
                    p2v = p2[:].rearrange("p (i f c) -> p i f c", i=ORDER, f=F, c=2)

                    # P1 = X5*C -> [xr*cr | xi*ci]
                    nc.gpsimd.tensor_mul(p1[:], x5[:], c[:])
                    # Sre[f,c] = sum_i P1[i,f,c]
                    nc.vector.tensor_reduce(
                        sre[:].rearrange("p (f c) -> p f c", f=F, c=2),
                        p1[:].rearrange("p (i f c) -> p f c i", i=ORDER, f=F, c=2),
                        axis=mybir.AxisListType.X,
                        op=add,
                    )
                    # P2 = [xi*cr | xr*ci]
                    nc.gpsimd.tensor_mul(
                        p2v[:, :, :, 0:1], x5v[:, :, :, 1:2], cv[:, :, :, 0:1]
                    )
                    nc.vector.tensor_mul(
                        p2v[:, :, :, 1:2], x5v[:, :, :, 0:1], cv[:, :, :, 1:2]
                    )
                    nc.vector.tensor_reduce(
                        sim[:].rearrange("p (f c) -> p f c", f=F, c=2),
                        p2[:].rearrange("p (i f c) -> p f c i", i=ORDER, f=F, c=2),
                        axis=mybir.AxisListType.X,
                        op=add,
                    )
                    srev = sre[:].rearrange("p (f c) -> p f c", f=F, c=2)
                    simv = sim[:].rearrange("p (f c) -> p f c", f=F, c=2)
                    accv = acc[:].rearrange("p (f c) -> p f c", f=F, c=2)
                    # re = Sre[even] - Sre[odd]; im = Sim[even] + Sim[odd]
                    nc.vector.tensor_tensor(
                        accv[:, :, 0:1], srev[:, :, 0:1], srev[:, :, 1:2], op=sub
                    )
                    nc.gpsimd.tensor_tensor(
                        accv[:, :, 1:2], simv[:, :, 0:1], simv[:, :, 1:2], op=add
                    )
                    # v = (1-alpha)*x0 ; x0 = tap LOOKAHEAD of X5
                    nc.scalar.activation(
                        v[:], x5[:, LOOKAHEAD * FC : (LOOKAHEAD + 1) * FC],
                        mybir.ActivationFunctionType.Copy,
                        scale=oma_sb[:, col : col + 1],
                    )
                    # out = alpha*acc + v
                    nc.vector.scalar_tensor_tensor(
                        ob[:, k * FC : (k + 1) * FC],
                        acc[:],
                        alpha_sb[:, col : col + 1],
                        v[:],
                        op0=mul,
                        op1=add,
                    )
                nc.sync.dma_start(
                    outb[b].rearrange("(k p) w -> p k w", p=128, k=NK), ob[:]
                )
    nc.compile()
    return nc


def _get_runner():
    """Build program + cached jitted shard_map executable (once per process)."""
    if "runner" in _CACHE:
        return _CACHE["runner"]

    import jax
    import jax.numpy as jnp
    from jax.sharding import Mesh, NamedSharding, PartitionSpec
    import concourse.mybir as mybir
    from concourse.bass2jax import (
        _bass_exec_p,
        install_neuronx_cc_hook,
        partition_id_tensor,
    )

    nc = _build_program()
    install_neuronx_cc_hook()

    partition_name = nc.partition_id_tensor.name if nc.partition_id_tensor else None
    in_names, out_names, out_avals = [], [], []
    for alloc in nc.m.functions[0].allocations:
        if not isinstance(alloc, mybir.MemoryLocationSet):
            continue
        name = alloc.memorylocations[0].name
        if alloc.kind == "ExternalInput":
            if name != partition_name:
                in_names.append(name)
        elif alloc.kind == "ExternalOutput":
            out_names.append(name)
            out_avals.append(
                jax.core.ShapedArray(tuple(alloc.tensor_shape), mybir.dt.np(alloc.dtype))
            )
    n_params = len(in_names)
    all_in_names = list(in_names) + list(out_names)
    if partition_name is not None:
        all_in_names.append(partition_name)

    def _body(*args):
        operands = list(args)
        if partition_name is not None:
            operands.append(partition_id_tensor())
        outs = _bass_exec_p.bind(
            *operands,
            out_avals=tuple(out_avals),
            in_names=tuple(all_in_names),
            out_names=tuple(out_names),
            lowering_input_output_aliases=(),
            sim_require_finite=True,
            sim_require_nnan=True,
            nc=nc,
        )
        return tuple(outs)

    devices = jax.devices()[:NCORES]
    mesh = Mesh(np.asarray(devices), ("core",))
    sh = NamedSharding(mesh, PartitionSpec("core"))
    n_outs = len(out_avals)
    sharded = jax.jit(
        jax.shard_map(
            _body,
            mesh=mesh,
            in_specs=(PartitionSpec("core"),) * (n_params + n_outs),
            out_specs=(PartitionSpec("core"),) * n_outs,
            check_vma=False,
        ),
        donate_argnums=tuple(range(n_params, n_params + n_outs)),
        keep_unused=True,
    )
    zeros_fn = jax.jit(
        lambda: jnp.zeros((B, TP, FC), jnp.float16), out_shardings=sh
    )
    _CACHE["runner"] = (sharded, zeros_fn, sh, in_names)
    return _CACHE["runner"]


class _Result:
    exec_time_ns = None
    profile_json = None
    results = None


def run_on_cores(spec, coefs, alpha, trace=False):
    """Full-input entry: shard, run on 8 cores, return (out_full, results_obj)."""
    import jax

    sharded, zeros_fn, sh, in_names = _get_runner()

    # host prep + async puts, cheapest tensor first so the wire starts early
    x_h = np.zeros((B, XROWS, FC), np.float16)
    x_h[:, LOOKAHEAD : LOOKAHEAD + T] = spec[:, 0, :, :F, :].reshape(B, T, FC)
    x_d = jax.device_put(x_h, sh)

    al = np.zeros((NCORES, BPC, TP), np.float32)
    al[:, :, :T] = np.asarray(alpha, np.float32).reshape(NCORES, BPC, T)
    om = np.zeros((NCORES, BPC, TP), np.float32)
    om[:, :, :T] = 1.0 - al[:, :, :T]
    # (core, b, k*128+p) -> (core*128 p, b*NK+k)
    alpha_h = np.ascontiguousarray(
        al.reshape(NCORES, BPC, NK, 128).transpose(0, 3, 1, 2)
    ).reshape(NCORES * 128, BPC * NK)
    oma_h = np.ascontiguousarray(
        om.reshape(NCORES, BPC, NK, 128).transpose(0, 3, 1, 2)
    ).reshape(NCORES * 128, BPC * NK)
    alpha_d = jax.device_put(alpha_h, sh)
    oma_d = jax.device_put(oma_h, sh)

    c_h = np.zeros((B, TP, W), np.float16)
    c_h[:, :T] = np.asarray(coefs).reshape(B, T, W)
    c_d = jax.device_put(c_h, sh)

    zeros_d = zeros_fn()

    ins = {"x_t": x_d, "c_t": c_d, "alpha_t": alpha_d, "oma_t": oma_d}
    out_arrs = sharded(*[ins[n] for n in in_names], zeros_d)
    outb = np.asarray(out_arrs[0])  # (32, 1024, 192) f16

    full = np.array(spec, dtype=np.float32, copy=True)  # f>=96 passthrough
    full[:, 0, :, :F, :] = outb[:, :T].reshape(B, T, F, 2)

    res = _Result()
    res.results = [{"outb": outb[c * BPC : (c + 1) * BPC]} for c in range(NCORES)]
    return full, res


def kernel(spec, coefs, alpha):
    spec = np.asarray(spec, dtype=np.float32)
    coefs = np.asarray(coefs, dtype=np.float32)
    alpha = np.asarray(alpha, dtype=np.float32)
    full, _ = run_on_cores(spec, coefs, alpha, trace=False)
    return full
